# revision 28
# baseline (speedup 1.0000x reference)
import numpy as np

import concourse.bacc as bacc
import concourse.mybir as mybir
from concourse import bass
from concourse.bass_utils import run_bass_kernel_spmd
from concourse.masks import make_identity, make_upper_triangular
from concourse.tile import TileContext

F32 = mybir.dt.float32
F32R = mybir.dt.float32r

NCORES = 8
H = 16
HPC = 2
HD = 64
E = 1024
L = 4096
LT = 512
NQT = L // LT
KT = 128
NKT = L // KT
ECH = E // 128
LROWS = L // NCORES
SCALE = 1.0 / 8.0


def r32(ap):
    return ap.bitcast(F32R)


def build_program(single_core=False, repeat=None, no_cc=False):
    nc = bacc.Bacc(
        trn_type="TRN2",
        target_bir_lowering=False,
        debug=False,
        num_devices=1 if single_core else NCORES,
    )

    xT = nc.dram_tensor("xT", [E, L], F32, kind="ExternalInput")
    wqkvT = nc.dram_tensor("wqkvT", [E, 6 * HD], F32, kind="ExternalInput")
    wprojT = nc.dram_tensor("wprojT", [E, E], F32, kind="ExternalInput")
    out = nc.dram_tensor("out", [LROWS, E], F32, kind="ExternalOutput")

    HLT = LT // 2
    a2a_in = nc.dram_tensor("a2a_in", [2, NCORES, HPC * HD, HLT], F32)
    a2a_out = nc.dram_tensor("a2a_out", [2, NCORES, HPC * HD, HLT], F32)
    rs_dram = nc.dram_tensor("rs_dram", [HPC * NQT, LT], F32)

    xT_r = xT[:].rearrange("(c p) l -> p c l", p=128)
    wqkvT_r = wqkvT[:].rearrange("(c p) w -> p c w", p=128)
    wprojT_r = wprojT[:].rearrange("(c p) o -> p c o", p=128)
    a2a_out_r = a2a_out[:].rearrange("s c p l -> s p c l")

    with TileContext(nc) as tc:
        with (
            tc.tile_pool(name="const", bufs=1) as const_pool,
            tc.tile_pool(name="store", bufs=1) as store_pool,
            tc.tile_pool(name="xt", bufs=2) as xt_pool,
            tc.tile_pool(name="p2", bufs=3) as p2_pool,
            tc.tile_pool(name="misc", bufs=2) as misc_pool,
            tc.tile_pool(name="ps", bufs=1, space="PSUM") as ps_pool,
            tc.tile_pool(name="psy", bufs=2, space="PSUM") as psy_pool,
        ):
            ident = const_pool.tile([128, 128], F32)
            make_identity(nc, ident[:])
            tri = const_pool.tile([128, 128], F32)
            make_upper_triangular(nc, tri[:], val=1.0, diag=True)

            wqkv_sb = const_pool.tile([128, ECH, 6 * HD], F32)
            wproj_sb = const_pool.tile([128, ECH, E], F32)
            if repeat is not None:
                nc.sync.dma_start(out=r32(wproj_sb[:]), in_=r32(wprojT_r))

            qT_sb = store_pool.tile([128, L], F32)
            kT_sb = store_pool.tile([128, L], F32)
            vT_sb = store_pool.tile([128, L], F32, tag="vT_sb")
            v_sb = store_pool.tile([128, HPC, NKT, HD + 1], F32)
            ones_src = const_pool.tile([128, HPC * NKT], F32)
            nc.vector.memset(ones_src[:], 1.0)
            nc.vector.tensor_copy(
                out=r32(v_sb[:, :, :, HD:HD + 1].rearrange("p a b c -> p (a b c)")),
                in_=ones_src[:],
            )

            dests = [qT_sb, kT_sb, vT_sb]

            def emit_all():
                def lt_units(lt):
                    xt = xt_pool.tile([128, ECH, LT], F32, name=f"xt{lt}",
                                      tag="xt")

                    def dma_unit():
                        for ec in range(ECH):
                            if lt == 0:
                                nc.sync.dma_start(out=r32(wqkv_sb[:, ec, :]),
                                                  in_=r32(wqkvT_r[:, ec, :]))
                            nc.sync.dma_start(
                                out=r32(xt[:, ec, :]),
                                in_=r32(xT_r[:, ec, lt * LT:(lt + 1) * LT]))
                    yield dma_unit

                    def g_unit(g):
                        ps = ps_pool.tile([128, 2, LT], F32,
                                          name=f"qkv{lt}{g}", tag="qkv", bufs=1)
                        for ec in range(ECH):
                            nc.tensor.matmul(
                                ps[:, 0, :],
                                lhsT=r32(wqkv_sb[:, ec, g * 128:(g + 1) * 128]),
                                rhs=r32(xt[:, ec, :]),
                                start=(ec == 0),
                                stop=(ec == ECH - 1),
                            )
                        nc.vector.tensor_copy(
                            out=r32(dests[g][:, lt * LT:(lt + 1) * LT]),
                            in_=ps[:, 0, :])
                    for g in range(3):
                        yield (lambda g=g: g_unit(g))

                    def tp_unit(h, j):
                        kt = lt * (LT // KT) + j
                        tp = ps_pool.tile([128, 2, LT], F32,
                                          name=f"tp{kt}{h}", tag="qkv", bufs=1)
                        nc.tensor.transpose(
                            tp[:, 0, 0:HD],
                            in_=vT_sb[h * HD:(h + 1) * HD, kt * KT:(kt + 1) * KT],
                            identity=ident[h * HD:(h + 1) * HD,
                                           h * HD:(h + 1) * HD],
                        )
                        nc.vector.tensor_copy(
                            out=r32(v_sb[:, h, kt, 0:HD]), in_=tp[:, 0, 0:HD])
                    for h in range(HPC):
                        for j in range(LT // KT):
                            yield (lambda h=h, j=j: tp_unit(h, j))

                def emit_scores(qt, kt):
                    c0 = max(0, kt * KT - qt * LT)
                    s2 = ps_pool.tile([128, 2, LT], F32,
                                      name=f"s{qt}_{kt}", tag="s", bufs=2)
                    for h in range(HPC):
                        nc.tensor.matmul(
                            s2[:, h, c0:LT],
                            lhsT=r32(kT_sb[h * HD:(h + 1) * HD,
                                           kt * KT:(kt + 1) * KT]),
                            rhs=r32(qT_sb[h * HD:(h + 1) * HD,
                                          qt * LT + c0:(qt + 1) * LT]),
                            start=True,
                            stop=True,
                        )
                    return s2

                def emit_kt(qt, kt, first, last, yps, s2):
                    c0 = max(0, kt * KT - qt * LT)
                    p2 = p2_pool.tile([128, 2, LT], F32,
                                      name=f"p{qt}_{kt}", tag="p2")
                    nc.scalar.activation(
                        out=r32(p2[:, :, c0:LT]),
                        in_=s2[:, :, c0:LT],
                        func=mybir.ActivationFunctionType.Exp,
                        scale=SCALE,
                    )
                    if kt * KT >= qt * LT:
                        for h in range(HPC):
                            nc.vector.tensor_mul(
                                r32(p2[:, h, c0:c0 + KT]),
                                p2[:, h, c0:c0 + KT],
                                tri[:, :],
                            )
                    for h in range(HPC):
                        nc.tensor.matmul(
                            yps[h][0:HD + 1, c0:LT],
                            lhsT=r32(v_sb[:, h, kt, 0:HD + 1]),
                            rhs=r32(p2[:, h, c0:LT]),
                            start=(kt == first),
                            stop=(kt == last),
                        )

                def emit_qt_tail(qt, yps, parts=None):
                    for h in range(HPC):
                        row = qt * HPC + h
                        ysb = misc_pool.tile([HD + 1, LT], F32,
                                             name=f"ysb{row}", tag="ysb")
                        if parts is not None:
                            nc.vector.tensor_add(ysb[:], yps[h][:], parts[h][:])
                        else:
                            nc.vector.tensor_copy(out=ysb[:], in_=yps[h][:])
                        rec = misc_pool.tile([HD + 1, LT], F32,
                                             name=f"rec{row}", tag="rec")
                        nc.vector.reciprocal(out=rec[HD:HD + 1, :],
                                             in_=ysb[HD:HD + 1, :])
                        nc.sync.dma_start(out=rs_dram[row, :],
                                          in_=rec[HD:HD + 1, :])
                        rb = misc_pool.tile([HD, LT], F32,
                                            name=f"rb{row}", tag="rb")
                        nc.sync.dma_start(
                            out=rb[:],
                            in_=rs_dram[row:row + 1, :].broadcast_to([HD, LT]))
                        yn = misc_pool.tile([HD, LT], F32,
                                            name=f"yn{row}", tag="yn")
                        nc.vector.tensor_mul(yn[:], ysb[0:HD, :], rb[:])
                        for s in range(2):
                            nc.sync.dma_start(
                                out=a2a_in[s, qt, h * HD:(h + 1) * HD, :],
                                in_=yn[:, s * HLT:(s + 1) * HLT])

                def attn_group(qt):
                    nkt = 4 * qt + 4
                    yps = [psy_pool.tile([HD + 1, LT], F32, tag="y",
                                         name=f"yps{qt}_{h}")
                           for h in range(HPC)]
                    s_next = emit_scores(qt, 0)
                    for kt in range(nkt):
                        s_cur = s_next
                        if kt + 1 < nkt:
                            s_next = emit_scores(qt, kt + 1)
                        emit_kt(qt, kt, 0, nkt - 1, yps, s_cur)
                    emit_qt_tail(qt, yps)

                def seq_lt(lt):
                    for u in lt_units(lt):
                        u()

                seq_lt(0)
                attn_group(0)
                seq_lt(1)
                if repeat is None:
                    nc.sync.dma_start(out=r32(wproj_sb[:]), in_=r32(wprojT_r))
                attn_group(1)
                for i in range(2, NQT):
                    seq_lt(i)
                    attn_group(i)

                ya_sb = store_pool.tile([128, ECH, LT], F32, name="ya_sb", tag="vT_sb")
                for half in range(2):
                    if single_core or no_cc:
                        nc.sync.dma_start(out=a2a_out[half], in_=a2a_in[half])
                    else:
                        nc.gpsimd.collective_compute(
                            "AllToAll",
                            mybir.AluOpType.bypass,
                            replica_groups=[list(range(NCORES))],
                            ins=[a2a_in[half]],
                            outs=[a2a_out[half]],
                        )
                    for fc in range(ECH):
                        nc.sync.dma_start(
                            out=r32(ya_sb[:, fc, half * HLT:(half + 1) * HLT]),
                            in_=r32(a2a_out_r[half, :, fc, :]))
                for rt in range(LROWS // 128):
                    ps = ps_pool.tile([128, 2, LT], F32, tag="qkv", bufs=1,
                                      name=f"proj{rt}")
                    for fc in range(ECH):
                        for nt in range(E // LT):
                            nc.tensor.matmul(
                                ps[:, nt, :],
                                lhsT=r32(ya_sb[:, fc, rt * 128:(rt + 1) * 128]),
                                rhs=r32(wproj_sb[:, fc, nt * LT:(nt + 1) * LT]),
                                start=(fc == 0),
                                stop=(fc == ECH - 1),
                            )
                    for nt in range(E // LT):
                        o_sb = misc_pool.tile([128, LT], F32, tag="o",
                                              name=f"o{rt}{nt}")
                        nc.scalar.copy(out=o_sb[:], in_=ps[:, nt, :])
                        nc.sync.dma_start(
                            out=out[rt * 128:(rt + 1) * 128,
                                    nt * LT:(nt + 1) * LT],
                            in_=o_sb[:],
                        )

            if repeat is not None:
                with tc.For_i(0, repeat, 1):
                    emit_all()
            else:
                emit_all()

    nc.compile()
    return nc


def shard_inputs(x, w_attn, w_proj):
    x = np.asarray(x, dtype=np.float32)
    w_attn = np.asarray(w_attn, dtype=np.float32)
    w_proj = np.asarray(w_proj, dtype=np.float32)
    xT = np.ascontiguousarray(x.reshape(L, E).T)
    wprojT = np.ascontiguousarray(w_proj.T)
    wq, wk, wv = w_attn[0:E], w_attn[E:2 * E], w_attn[2 * E:3 * E]
    in_maps = []
    for c in range(NCORES):
        h0, h1 = HPC * c, HPC * c + 1
        cols = np.concatenate([
            wq[h0 * HD:(h0 + 1) * HD], wq[h1 * HD:(h1 + 1) * HD],
            wk[h0 * HD:(h0 + 1) * HD], wk[h1 * HD:(h1 + 1) * HD],
            wv[h0 * HD:(h0 + 1) * HD], wv[h1 * HD:(h1 + 1) * HD],
        ], axis=0)
        in_maps.append({
            "xT": xT,
            "wqkvT": np.ascontiguousarray(cols.T),
            "wprojT": wprojT,
        })
    return in_maps


_NC_CACHE = None


def get_program():
    global _NC_CACHE
    if _NC_CACHE is None:
        _NC_CACHE = build_program()
    return _NC_CACHE


def kernel(x, w_attn, w_proj):
    nc = get_program()
    in_maps = shard_inputs(x, w_attn, w_proj)
    res = run_bass_kernel_spmd(nc, in_maps, list(range(NCORES)))
    out = np.concatenate([res.results[c]["out"] for c in range(NCORES)], axis=0)
    return out.reshape(1, L, E).astype(np.float32)
